# revision 20
# baseline (speedup 1.0000x reference)
"""CaptionEmbedder kernel for Trainium2 (Bass), 8-core data-parallel.

Semantics (matching the reference):
    ent_idx  = clamp-to-49 of (caption_indices - 32000)   (oob -> 49)
    word_idx = caption_indices if < 32000 else pad_token
    out[b,l] = entities_encoded[b, ent_idx]  if caption_masks[b,l,0] == 1
               else word_embedding[word_idx]

Strategy: shard the batch dim (8 batches/core). The host concatenates the
core's entity shard [400, 512] onto the word table -> combined table
[32400, 512] per core, in bf16 (rel err ~4e-3, tolerance 2e-2).

Key structural insight: ~50% of tokens are masked entity tokens, and the
input range guarantees almost all of them clamp to entity slot 49 -- i.e.
per batch they all read ONE table row.  Indirect-DMA descriptor generation
costs ~1.1 us of Q7 time per instruction (fixed-cost dominated), so we
split tokens on the host:

  tail (masked & ent==49): filled by a single HWDGE DMA straight
       DRAM->DRAM with a stride-0 source AP -- table row V+50b+49
       broadcast 128x into out column 7+b.  Zero Q7 desc-gen, zero input
       dependencies: it issues the moment the preamble barrier drops.
  head (everything else, ~800 tokens): host-permuted to the front,
       gathered with per-column native indirect DMAs (7 columns of 128
       rows -- the only offset shape the HW SWDGE ucode handles), staged
       through SBUF and stored with per-column HWDGE DMAs that pipeline
       behind the gathers.

The host computes all row indices in numpy (pure int math on host-visible
inputs), so the device never touches the index arithmetic.  No extended
gpsimd library is needed (a LOAD_LIB costs ~9 us of Q7 stall).

Output layout [128, 15, 512]: cols 0-6 head slot j -> [j%128, j//128],
cols 7-14 tail slot (b, k) -> [k, 7+b].  Host unpermutes.
"""

import os
import sys
from functools import lru_cache

import numpy as np

for _p in ("/opt/trn_rl_repo",):
    if _p not in sys.path:
        sys.path.insert(0, _p)

# Problem shapes (hardcoded per contest contract).
V = 32000          # vocab size
B = 64             # batch
L = 200            # caption length
N_ENT = 50         # entities per batch
D = 512            # embedding dim
N_CORES = 8
B_LOC = B // N_CORES            # 8 batches per core
TOK = B_LOC * L                 # 1600 tokens per core
P = 128                         # SBUF partitions
TBL = V + B_LOC * N_ENT         # 32400 rows in combined table

HCOLS = 7                       # head columns
HLASTP = 96                     # valid partitions in the last head column
HCAP = (HCOLS - 1) * P + HLASTP  # 864 >= W ~ 800+-30 (max observed 836)
TCOLS = B_LOC                   # one tail column per local batch (cap 128)


def _groups(head_cols):
    """Split head columns into gather groups (amortize desc-gen fixed cost
    while keeping store/transfer overlap). CAPEMB_GROUPS="4,3" overrides."""
    spec = os.environ.get("CAPEMB_GROUPS")
    if spec:
        sizes = [int(x) for x in spec.split(",")]
    else:
        # ONE column per indirect DMA: the native SWDGE ucode mishandles
        # multi-column offset APs on real HW (NaN + DMA-timeout sems, even
        # though CoreSim accepts them). 128 offsets per instruction is the
        # hardware-validated shape.
        sizes = [1] * head_cols
    sizes = [s for s in sizes if s > 0]
    assert sum(sizes) == head_cols, (sizes, head_cols)
    out, c0 = [], 0
    for s in sizes:
        out.append((c0, s))
        c0 += s
    return tuple(out)


def _indirect_gather(gpsimd, out, in_, offset_ap_with_axis):
    """indirect_dma_start minus the SBUF-destination restriction: allows the
    gather to write DRAM directly (in_ DRAM -> out DRAM or SBUF)."""
    from concourse import mybir

    offset_ap = offset_ap_with_axis.ap
    offset_axis = offset_ap_with_axis.axis
    assert isinstance(in_.offset, int) and in_.offset == 0
    out_l = gpsimd.lower_ap_dma(out, for_indirect_dma=True)
    in_l = gpsimd.lower_ap_dma(in_, for_indirect_dma=True)
    assert len(in_l) == 1 and len(out_l) == 1
    off_l = gpsimd.lower_ap_dma(offset_ap)
    assert len(off_l) == 1
    in_l.append(off_l[0])

    ap_shape = in_.shape
    coef = 1
    for i in range(offset_axis + 1, len(ap_shape)):
        coef *= ap_shape[i]
    in_l[0].dynamic_ap_info = mybir.DynamicAccessPatternInfo(
        c=0,
        actual_ap=out.ap,
        indirect_dim_max_index=ap_shape[offset_axis],
        offset_expr=[
            mybir.DynamicAccessPatternOffsetExpr(
                coef=coef,
                aff_expr=mybir.DynamicAccessPatternOffsetExprAffExpr(
                    kind="IndirectArgId", arg_id=1
                ),
            )
        ],
    )
    return gpsimd.add_instruction(
        mybir.InstDMACopy(
            name=gpsimd.bass.get_next_instruction_name(),
            queue="qPoolDynamic",
            mode="Copy",
            ins=in_l,
            outs=out_l,
            oob_is_err=True,
            cce_op=mybir.AluOpType.bypass,
        )
    )


def _build_common(nc, head_cols, sbuf_stage):
    """Emit the kernel body: tail broadcast + head gather (+ stores)."""
    import concourse.bass as bass
    from concourse import mybir
    from concourse.ap import AP

    i32 = mybir.dt.int32
    bf16 = mybir.dt.bfloat16

    out_cols = head_cols + TCOLS
    tbl_h = nc.dram_tensor("table", [TBL, D], bf16, kind="ExternalInput")
    idx_h = nc.dram_tensor("idx", [P, head_cols], i32, kind="ExternalInput")
    out_h = nc.dram_tensor("out", [P, out_cols, D], bf16, kind="ExternalOutput")
    tbl_ap = tbl_h.ap()
    out_ap = out_h.ap()

    idx_sb = nc.alloc_sbuf_tensor("idx_sb", [P, head_cols], i32).ap()
    emb3 = (
        nc.alloc_sbuf_tensor("emb", [P, head_cols, D], bf16).ap()
        if sbuf_stage
        else None
    )

    groups = _groups(head_cols)
    sem_idx = nc.alloc_semaphore("sem_idx")
    sem_t = nc.alloc_semaphore("sem_t")
    sem_gs = [nc.alloc_semaphore(f"sem_g{k}") for k in range(len(groups))]
    sem_s = nc.alloc_semaphore("sem_s")

    # table rows V+49, V+99, ..., V+50*7+49 broadcast 128x each:
    # src dims (128 reps, 8 batches, 512) pair with dst dims of
    # out[:, head_cols:, :].
    ent49 = AP(
        tensor=tbl_h,
        offset=(V + N_ENT - 1) * D,
        ap=[[0, P], [N_ENT * D, B_LOC], [1, D]],
    )

    # valid partitions of the last head column (896 slots would waste
    # desc-gen + transfer on ~60 always-dummy rows; W maxes out at ~840)
    vp_last = (
        HLASTP if head_cols == HCOLS else P
    )

    with nc.Block() as block:

        @block.gpsimd
        def _(gpsimd):
            # SWDGE self-load of the index tile: same-engine sem wait skips
            # the cross-engine wake hop before the first gather
            gpsimd.dma_start(out=idx_sb, in_=idx_h.ap()[:, :]).then_inc(
                sem_idx, 16
            )
            gpsimd.wait_ge(sem_idx, 16)
            for k, (c0, g) in enumerate(groups):
                if g == 1 and sbuf_stage:
                    vp = vp_last if c0 == head_cols - 1 else P
                    # stock path with a SQUEEZED 2-dim out AP [vp, 512]:
                    # the exact shape validated on HW. A singleton middle
                    # dim ([128, 1, 512]) or multi-column out both produce
                    # NaN + DMA-timeout sems on real silicon.
                    gpsimd.indirect_dma_start(
                        out=emb3[0:vp, c0, :],
                        out_offset=None,
                        in_=tbl_ap[:, :],
                        in_offset=bass.IndirectOffsetOnAxis(
                            ap=idx_sb[0:vp, c0 : c0 + 1], axis=0
                        ),
                    ).then_inc(sem_gs[k], 16)
                    continue
                if sbuf_stage:
                    gpsimd.indirect_dma_start(
                        out=emb3[:, c0 : c0 + g, :],
                        out_offset=None,
                        in_=tbl_ap[:, :],
                        in_offset=bass.IndirectOffsetOnAxis(
                            ap=idx_sb[:, c0 : c0 + g], axis=0
                        ),
                    ).then_inc(sem_gs[k], 16)
                    continue
                _indirect_gather(
                    gpsimd,
                    out_ap[:, c0 : c0 + g, :],
                    tbl_ap[:, :],
                    bass.IndirectOffsetOnAxis(
                        ap=idx_sb[:, c0 : c0 + g], axis=0
                    ),
                ).then_inc(sem_gs[k], 16)

        @block.sync
        def _(sync):
            # tail broadcast: no dependencies, issues immediately
            sync.dma_start(
                out=out_ap[:, head_cols:out_cols, :], in_=ent49
            ).then_inc(sem_t, 16)
            if sbuf_stage:
                for k, (c0, g) in enumerate(groups):
                    sync.wait_ge(sem_gs[k], 16)
                    vp = vp_last if c0 + g == head_cols else P
                    sync.dma_start(
                        out=out_ap[0:vp, c0 : c0 + g, :],
                        in_=emb3[0:vp, c0 : c0 + g, :],
                    ).then_inc(sem_s, 16)
                sync.wait_ge(sem_s, 16 * len(groups))
            else:
                for k in range(len(groups)):
                    sync.wait_ge(sem_gs[k], 16)
            sync.wait_ge(sem_t, 16)

    # Block exit emitted an all-engine barrier; reset our semaphores so the
    # NEFF is re-executable.
    for s in (sem_idx, sem_t, *sem_gs, sem_s):
        nc.gpsimd.sem_clear(s)

    nc.compile()
    return nc


def _sbuf_stage():
    # Direct-to-DRAM indirect gather (CAPEMB_D2D=1) passes CoreSim but
    # crashes real hardware (known-buggy DRAM->DRAM indirect path), so the
    # default stages through SBUF with per-group HWDGE stores.
    return not bool(os.environ.get("CAPEMB_D2D"))


def _use_ext():
    # CAPEMB_EXT=1: head gather via the extended dma_gather ucode (mlp
    # library) with an eager LOAD_LIB overlapping the idx load.
    return bool(os.environ.get("CAPEMB_EXT"))


# extended-gather chunks: (start_token, n); n multiples of 128 keep the
# global token -> [t%128, t//128] slot map; queue k -> Q7 core pair k, so
# the four chunks' descriptor generation runs concurrently.
EXT_CHUNKS = ((0, 256), (256, 256), (512, 256), (768, 128))
assert sum(n for _, n in EXT_CHUNKS) == HCOLS * P


@lru_cache(maxsize=1)
def _build_ext():
    import concourse.bacc as bacc
    from concourse import mybir, library_config
    from concourse.ap import AP

    i16 = mybir.dt.int16
    bf16 = mybir.dt.bfloat16
    head_cols = HCOLS
    out_cols = head_cols + TCOLS

    nc = bacc.Bacc(
        "TRN2", target_bir_lowering=False, debug=False, num_swdge_queues=4
    )
    tbl_h = nc.dram_tensor("table", [TBL, D], bf16, kind="ExternalInput")
    idx_h = nc.dram_tensor("idx", [P, HCOLS * P // 16], i16, kind="ExternalInput")
    out_h = nc.dram_tensor("out", [P, out_cols, D], bf16, kind="ExternalOutput")
    tbl_ap = tbl_h.ap()
    out_ap = out_h.ap()

    idx_sb = nc.alloc_sbuf_tensor("idx_sb", [P, HCOLS * P // 16], i16).ap()
    emb3 = nc.alloc_sbuf_tensor("emb", [P, head_cols, D], bf16).ap()

    sem_idx = nc.alloc_semaphore("sem_idx")
    sem_t = nc.alloc_semaphore("sem_t")
    sem_gs = [nc.alloc_semaphore(f"sem_g{k}") for k in range(len(EXT_CHUNKS))]
    sem_s = nc.alloc_semaphore("sem_s")

    ent49 = AP(
        tensor=tbl_h,
        offset=(V + N_ENT - 1) * D,
        ap=[[0, P], [N_ENT * D, B_LOC], [1, D]],
    )

    with nc.Block() as block:

        @block.scalar
        def _(scalar):
            scalar.dma_start(out=idx_sb, in_=idx_h.ap()[:, :]).then_inc(
                sem_idx, 16
            )

        @block.gpsimd
        def _(gpsimd):
            # eager library load: the ~9 us Q7 ucode install overlaps the
            # idx DMA instead of starting after it
            gpsimd.load_library(library_config.mlp)
            gpsimd.wait_ge(sem_idx, 16)
            for k, (t0, n) in enumerate(EXT_CHUNKS):
                c0 = t0 // P
                gpsimd.dma_gather(
                    out_ap=emb3[:, c0 : c0 + n // P, :],
                    in_ap=tbl_ap[:, :],
                    idxs_ap=idx_sb[:, t0 // 16 : (t0 + n) // 16],
                    num_idxs=n,
                    num_idxs_reg=n,
                    elem_size=D,
                    queue_num=k,
                ).then_inc(sem_gs[k], 16)

        @block.sync
        def _(sync):
            sync.dma_start(
                out=out_ap[:, head_cols:out_cols, :], in_=ent49
            ).then_inc(sem_t, 16)
            for k, (t0, n) in enumerate(EXT_CHUNKS):
                sync.wait_ge(sem_gs[k], 16)
                c0 = t0 // P
                sync.dma_start(
                    out=out_ap[:, c0 : c0 + n // P, :],
                    in_=emb3[:, c0 : c0 + n // P, :],
                ).then_inc(sem_s, 16)
            sync.wait_ge(sem_s, 16 * len(EXT_CHUNKS))
            sync.wait_ge(sem_t, 16)

    for s in (sem_idx, sem_t, *sem_gs, sem_s):
        nc.gpsimd.sem_clear(s)

    nc.compile()
    return nc


@lru_cache(maxsize=2)
def _build(sbuf_stage):
    import concourse.bacc as bacc

    nc = bacc.Bacc("TRN2", target_bir_lowering=False, debug=False)
    return _build_common(nc, HCOLS, sbuf_stage)


@lru_cache(maxsize=2)
def _build_general(sbuf_stage):
    """Fallback for pathological inputs where head/tail capacities overflow:
    all 1600 tokens go through the head gather (13 columns)."""
    import concourse.bacc as bacc

    nc = bacc.Bacc("TRN2", target_bir_lowering=False, debug=False)
    return _build_common(nc, -(-TOK // P), sbuf_stage)


def _to_bf16(a):
    import ml_dtypes

    return np.asarray(a).astype(ml_dtypes.bfloat16)


def _route(ci, cm, pad, head_cols):
    """Per-core token routing. Returns (head_rows[cap padded], src_slot[1600])
    or None if capacities overflow."""
    hcap = HCAP if head_cols == HCOLS else head_cols * P
    ent = ci - V
    entc = np.where((ent < 0) | (ent >= N_ENT), N_ENT - 1, ent)
    word = np.where(ci >= V, pad, ci)
    ent_base = V + N_ENT * np.arange(B_LOC)[:, None]
    rows = np.where(cm == 1, ent_base + entc, word)      # [8, 200]
    is_tail = (cm == 1) & (entc == N_ENT - 1)            # [8, 200]

    rows_f = rows.reshape(TOK)
    src_slot = np.empty(TOK, dtype=np.int64)
    t_base = np.arange(L)
    head_toks = []
    for b in range(B_LOC):
        tl = t_base[is_tail[b]]
        if len(tl) > P:  # overflow -> route excess to head
            head_toks.extend((b * L + tl[P:]).tolist())
            tl = tl[:P]
        src_slot[b * L + tl] = (head_cols + b) * P + np.arange(len(tl))
        head_toks.extend((b * L + t_base[~is_tail[b]]).tolist())
    head_toks = np.asarray(sorted(head_toks), dtype=np.int64)
    if len(head_toks) > hcap:
        return None
    src_slot[head_toks] = np.arange(len(head_toks))
    head_rows = np.zeros(head_cols * P, dtype=np.int32)
    head_rows[: len(head_toks)] = rows_f[head_toks]
    return head_rows, src_slot


def _shard_inputs(caption_indices, entities_encoded, word_embedding,
                  pad_token, caption_masks, head_cols):
    ci = np.asarray(caption_indices, dtype=np.int64)          # [64, 200]
    cm = np.asarray(caption_masks, dtype=np.int64)[:, :, 0]   # [64, 200]
    we = _to_bf16(word_embedding)                             # [32000, 512]
    ee = _to_bf16(entities_encoded)                           # [64, 50, 512]
    pad = int(pad_token)

    in_maps, slot_maps = [], []
    for i in range(N_CORES):
        sl = slice(i * B_LOC, (i + 1) * B_LOC)
        routed = _route(ci[sl], cm[sl], pad, head_cols)
        if routed is None:
            return None
        head_rows, src_slot = routed
        if _use_ext() and head_cols == HCOLS:
            # 16-wrap int16 layout for dma_gather: token j -> [j%16, j//16],
            # replicated across the 8 Q7 core groups (128 partitions)
            idx_np = np.ascontiguousarray(
                np.tile(head_rows.astype(np.int16).reshape(-1, 16).T, (8, 1))
            )
        else:
            idx_np = np.ascontiguousarray(
                head_rows.reshape(head_cols, P).T
            )                                                 # [128, hcols]
        tbl = np.concatenate([we, ee[sl].reshape(-1, D)], axis=0)
        in_maps.append({"table": np.ascontiguousarray(tbl), "idx": idx_np})
        slot_maps.append(src_slot)
    return in_maps, slot_maps


LAST_RESULTS = None  # BassKernelResults of the most recent run (for test.py)


def kernel(caption_indices, entities_encoded, word_embedding, pad_token,
           caption_masks):
    global LAST_RESULTS
    from concourse.bass_utils import run_bass_kernel_spmd

    head_cols = HCOLS
    sharded = _shard_inputs(caption_indices, entities_encoded,
                            word_embedding, pad_token, caption_masks,
                            head_cols)
    if sharded is None:
        head_cols = -(-TOK // P)
        sharded = _shard_inputs(caption_indices, entities_encoded,
                                word_embedding, pad_token, caption_masks,
                                head_cols)
        nc = _build_general(_sbuf_stage())
    elif _use_ext():
        nc = _build_ext()
    else:
        nc = _build(_sbuf_stage())
    in_maps, slot_maps = sharded

    res = run_bass_kernel_spmd(
        nc,
        in_maps,
        list(range(N_CORES)),
        trace=bool(os.environ.get("CAPEMB_TRACE")),
    )
    LAST_RESULTS = res
    out = np.empty((B, L, D), dtype=np.float32)
    out_cols = head_cols + TCOLS
    for i in range(N_CORES):
        toks = np.transpose(res.results[i]["out"], (1, 0, 2)).reshape(
            out_cols * P, D
        )
        out[i * B_LOC : (i + 1) * B_LOC] = (
            toks[slot_maps[i]].astype(np.float32).reshape(B_LOC, L, D)
        )
    return out


# revision 27
# speedup vs baseline: 1.2843x; 1.2843x over previous
"""CaptionEmbedder kernel for Trainium2 (Bass), 8-core data-parallel.

Semantics (matching the reference):
    ent_idx  = clamp-to-49 of (caption_indices - 32000)   (oob -> 49)
    word_idx = caption_indices if < 32000 else pad_token
    out[b,l] = entities_encoded[b, ent_idx]  if caption_masks[b,l,0] == 1
               else word_embedding[word_idx]

Strategy: shard the batch dim (8 batches/core). The host concatenates the
core's entity shard [400, 512] onto the word table -> combined table
[32400, 512] per core, in bf16 (rel err ~4e-3, tolerance 2e-2).

Key structural insight: ~50% of tokens are masked entity tokens, and the
input range guarantees almost all of them clamp to entity slot 49 -- i.e.
per batch they all read ONE table row.  Indirect-DMA descriptor generation
costs ~1.1 us of Q7 time per instruction (fixed-cost dominated), so we
split tokens on the host:

  tail (masked & ent==49): filled by a single HWDGE DMA straight
       DRAM->DRAM with a stride-0 source AP -- table row V+50b+49
       broadcast 128x into out column 7+b.  Zero Q7 desc-gen, zero input
       dependencies: it issues the moment the preamble barrier drops.
  head (everything else, ~800 tokens): host-permuted to the front,
       gathered with per-column native indirect DMAs (7 columns of 128
       rows -- the only offset shape the HW SWDGE ucode handles), staged
       through SBUF and stored with per-column HWDGE DMAs that pipeline
       behind the gathers.

The host computes all row indices in numpy (pure int math on host-visible
inputs), so the device never touches the index arithmetic.  No extended
gpsimd library is needed (a LOAD_LIB costs ~9 us of Q7 stall).

Output layout [128, 15, 512]: cols 0-6 head slot j -> [j%128, j//128],
cols 7-14 tail slot (b, k) -> [k, 7+b].  Host unpermutes.
"""

import os
import sys
from functools import lru_cache

import numpy as np

for _p in ("/opt/trn_rl_repo",):
    if _p not in sys.path:
        sys.path.insert(0, _p)

# Problem shapes (hardcoded per contest contract).
V = 32000          # vocab size
B = 64             # batch
L = 200            # caption length
N_ENT = 50         # entities per batch
D = 512            # embedding dim
N_CORES = 8
B_LOC = B // N_CORES            # 8 batches per core
TOK = B_LOC * L                 # 1600 tokens per core
P = 128                         # SBUF partitions
TBL = V + B_LOC * N_ENT         # 32400 rows in combined table

HCOLS = 7                       # head columns (capacity 896 >= W ~ 800+-30)
TCOLS = B_LOC                   # one tail column per local batch (cap 128)


def _groups(head_cols):
    """Split head columns into gather groups (amortize desc-gen fixed cost
    while keeping store/transfer overlap). CAPEMB_GROUPS="4,3" overrides."""
    spec = os.environ.get("CAPEMB_GROUPS")
    if spec:
        sizes = [int(x) for x in spec.split(",")]
    else:
        # ONE column per indirect DMA: the native SWDGE ucode mishandles
        # multi-column offset APs on real HW (NaN + DMA-timeout sems, even
        # though CoreSim accepts them). 128 offsets per instruction is the
        # hardware-validated shape.
        sizes = [1] * head_cols
    sizes = [s for s in sizes if s > 0]
    assert sum(sizes) == head_cols, (sizes, head_cols)
    out, c0 = [], 0
    for s in sizes:
        out.append((c0, s))
        c0 += s
    return tuple(out)


def _indirect_gather(gpsimd, out, in_, offset_ap_with_axis):
    """indirect_dma_start minus the SBUF-destination restriction: allows the
    gather to write DRAM directly (in_ DRAM -> out DRAM or SBUF)."""
    from concourse import mybir

    offset_ap = offset_ap_with_axis.ap
    offset_axis = offset_ap_with_axis.axis
    assert isinstance(in_.offset, int) and in_.offset == 0
    out_l = gpsimd.lower_ap_dma(out, for_indirect_dma=True)
    in_l = gpsimd.lower_ap_dma(in_, for_indirect_dma=True)
    assert len(in_l) == 1 and len(out_l) == 1
    off_l = gpsimd.lower_ap_dma(offset_ap)
    assert len(off_l) == 1
    in_l.append(off_l[0])

    ap_shape = in_.shape
    coef = 1
    for i in range(offset_axis + 1, len(ap_shape)):
        coef *= ap_shape[i]
    in_l[0].dynamic_ap_info = mybir.DynamicAccessPatternInfo(
        c=0,
        actual_ap=out.ap,
        indirect_dim_max_index=ap_shape[offset_axis],
        offset_expr=[
            mybir.DynamicAccessPatternOffsetExpr(
                coef=coef,
                aff_expr=mybir.DynamicAccessPatternOffsetExprAffExpr(
                    kind="IndirectArgId", arg_id=1
                ),
            )
        ],
    )
    return gpsimd.add_instruction(
        mybir.InstDMACopy(
            name=gpsimd.bass.get_next_instruction_name(),
            queue="qPoolDynamic",
            mode="Copy",
            ins=in_l,
            outs=out_l,
            oob_is_err=True,
            cce_op=mybir.AluOpType.bypass,
        )
    )


def _build_common(nc, head_cols, sbuf_stage):
    """Emit the kernel body: tail broadcast + head gather (+ stores)."""
    import concourse.bass as bass
    from concourse import mybir
    from concourse.ap import AP

    i32 = mybir.dt.int32
    bf16 = mybir.dt.bfloat16

    out_cols = head_cols + TCOLS
    tbl_h = nc.dram_tensor("table", [TBL, D], bf16, kind="ExternalInput")
    idx_h = nc.dram_tensor("idx", [P, head_cols], i32, kind="ExternalInput")
    out_h = nc.dram_tensor("out", [P, out_cols, D], bf16, kind="ExternalOutput")
    tbl_ap = tbl_h.ap()
    out_ap = out_h.ap()

    idx_sb = nc.alloc_sbuf_tensor("idx_sb", [P, head_cols], i32).ap()
    emb3 = (
        nc.alloc_sbuf_tensor("emb", [P, head_cols, D], bf16).ap()
        if sbuf_stage
        else None
    )

    groups = _groups(head_cols)
    sem_idx = nc.alloc_semaphore("sem_idx")
    sem_t = nc.alloc_semaphore("sem_t")
    sem_gs = [nc.alloc_semaphore(f"sem_g{k}") for k in range(len(groups))]
    sem_s = nc.alloc_semaphore("sem_s")

    # table rows V+49, V+99, ..., V+50*7+49 broadcast 128x each:
    # src dims (128 reps, 8 batches, 512) pair with dst dims of
    # out[:, head_cols:, :].
    ent49 = AP(
        tensor=tbl_h,
        offset=(V + N_ENT - 1) * D,
        ap=[[0, P], [N_ENT * D, B_LOC], [1, D]],
    )

    with nc.Block() as block:

        @block.scalar
        def _(scalar):
            scalar.dma_start(out=idx_sb, in_=idx_h.ap()[:, :]).then_inc(
                sem_idx, 16
            )

        @block.gpsimd
        def _(gpsimd):
            gpsimd.wait_ge(sem_idx, 16)
            for k, (c0, g) in enumerate(groups):
                if g == 1 and sbuf_stage:
                    # stock path with a SQUEEZED 2-dim out AP [128, 512]:
                    # the exact shape validated on HW. A singleton middle
                    # dim ([128, 1, 512]) or multi-column out both produce
                    # NaN + DMA-timeout sems on real silicon.
                    gpsimd.indirect_dma_start(
                        out=emb3[:, c0, :],
                        out_offset=None,
                        in_=tbl_ap[:, :],
                        in_offset=bass.IndirectOffsetOnAxis(
                            ap=idx_sb[:, c0 : c0 + 1], axis=0
                        ),
                    ).then_inc(sem_gs[k], 16)
                    continue
                if sbuf_stage:
                    gpsimd.indirect_dma_start(
                        out=emb3[:, c0 : c0 + g, :],
                        out_offset=None,
                        in_=tbl_ap[:, :],
                        in_offset=bass.IndirectOffsetOnAxis(
                            ap=idx_sb[:, c0 : c0 + g], axis=0
                        ),
                    ).then_inc(sem_gs[k], 16)
                    continue
                _indirect_gather(
                    gpsimd,
                    out_ap[:, c0 : c0 + g, :],
                    tbl_ap[:, :],
                    bass.IndirectOffsetOnAxis(
                        ap=idx_sb[:, c0 : c0 + g], axis=0
                    ),
                ).then_inc(sem_gs[k], 16)

        @block.sync
        def _(sync):
            # tail broadcast: no dependencies, issues immediately
            sync.dma_start(
                out=out_ap[:, head_cols:out_cols, :], in_=ent49
            ).then_inc(sem_t, 16)
            if sbuf_stage:
                for k, (c0, g) in enumerate(groups):
                    sync.wait_ge(sem_gs[k], 16)
                    sync.dma_start(
                        out=out_ap[:, c0 : c0 + g, :],
                        in_=emb3[:, c0 : c0 + g, :],
                    ).then_inc(sem_s, 16)
                sync.wait_ge(sem_s, 16 * len(groups))
            else:
                for k in range(len(groups)):
                    sync.wait_ge(sem_gs[k], 16)
            sync.wait_ge(sem_t, 16)

    # Block exit emitted an all-engine barrier; reset our semaphores so the
    # NEFF is re-executable.
    for s in (sem_idx, sem_t, *sem_gs, sem_s):
        nc.gpsimd.sem_clear(s)

    nc.compile()
    return nc


def _sbuf_stage():
    # Direct-to-DRAM indirect gather (CAPEMB_D2D=1) passes CoreSim but
    # crashes real hardware (known-buggy DRAM->DRAM indirect path), so the
    # default stages through SBUF with per-group HWDGE stores.
    return not bool(os.environ.get("CAPEMB_D2D"))


def _use_ext():
    # CAPEMB_EXT=1: head gather via the extended dma_gather ucode (mlp
    # library) with an eager LOAD_LIB overlapping the idx load.
    return bool(os.environ.get("CAPEMB_EXT"))


# extended-gather chunks: (start_token, n); n multiples of 128 keep the
# global token -> [t%128, t//128] slot map; queue k -> Q7 core pair k, so
# the four chunks' descriptor generation runs concurrently.
EXT_CHUNKS = ((0, 256), (256, 256), (512, 256), (768, 128))
assert sum(n for _, n in EXT_CHUNKS) == HCOLS * P


@lru_cache(maxsize=1)
def _build_ext():
    import concourse.bacc as bacc
    from concourse import mybir, library_config
    from concourse.ap import AP

    i16 = mybir.dt.int16
    bf16 = mybir.dt.bfloat16
    head_cols = HCOLS
    out_cols = head_cols + TCOLS

    nc = bacc.Bacc(
        "TRN2", target_bir_lowering=False, debug=False, num_swdge_queues=4
    )
    tbl_h = nc.dram_tensor("table", [TBL, D], bf16, kind="ExternalInput")
    idx_h = nc.dram_tensor("idx", [P, HCOLS * P // 16], i16, kind="ExternalInput")
    out_h = nc.dram_tensor("out", [P, out_cols, D], bf16, kind="ExternalOutput")
    tbl_ap = tbl_h.ap()
    out_ap = out_h.ap()

    idx_sb = nc.alloc_sbuf_tensor("idx_sb", [P, HCOLS * P // 16], i16).ap()
    emb3 = nc.alloc_sbuf_tensor("emb", [P, head_cols, D], bf16).ap()

    sem_idx = nc.alloc_semaphore("sem_idx")
    sem_t = nc.alloc_semaphore("sem_t")
    sem_gs = [nc.alloc_semaphore(f"sem_g{k}") for k in range(len(EXT_CHUNKS))]
    sem_s = nc.alloc_semaphore("sem_s")

    ent49 = AP(
        tensor=tbl_h,
        offset=(V + N_ENT - 1) * D,
        ap=[[0, P], [N_ENT * D, B_LOC], [1, D]],
    )

    with nc.Block() as block:

        @block.scalar
        def _(scalar):
            scalar.dma_start(out=idx_sb, in_=idx_h.ap()[:, :]).then_inc(
                sem_idx, 16
            )

        @block.gpsimd
        def _(gpsimd):
            # eager library load: the ~9 us Q7 ucode install overlaps the
            # idx DMA instead of starting after it
            gpsimd.load_library(library_config.mlp)
            gpsimd.wait_ge(sem_idx, 16)
            for k, (t0, n) in enumerate(EXT_CHUNKS):
                c0 = t0 // P
                gpsimd.dma_gather(
                    out_ap=emb3[:, c0 : c0 + n // P, :],
                    in_ap=tbl_ap[:, :],
                    idxs_ap=idx_sb[:, t0 // 16 : (t0 + n) // 16],
                    num_idxs=n,
                    num_idxs_reg=n,
                    elem_size=D,
                    queue_num=k,
                ).then_inc(sem_gs[k], 16)

        @block.sync
        def _(sync):
            sync.dma_start(
                out=out_ap[:, head_cols:out_cols, :], in_=ent49
            ).then_inc(sem_t, 16)
            for k, (t0, n) in enumerate(EXT_CHUNKS):
                sync.wait_ge(sem_gs[k], 16)
                c0 = t0 // P
                sync.dma_start(
                    out=out_ap[:, c0 : c0 + n // P, :],
                    in_=emb3[:, c0 : c0 + n // P, :],
                ).then_inc(sem_s, 16)
            sync.wait_ge(sem_s, 16 * len(EXT_CHUNKS))
            sync.wait_ge(sem_t, 16)

    for s in (sem_idx, sem_t, *sem_gs, sem_s):
        nc.gpsimd.sem_clear(s)

    nc.compile()
    return nc


@lru_cache(maxsize=2)
def _build(sbuf_stage):
    import concourse.bacc as bacc

    nc = bacc.Bacc("TRN2", target_bir_lowering=False, debug=False)
    return _build_common(nc, HCOLS, sbuf_stage)


@lru_cache(maxsize=2)
def _build_general(sbuf_stage):
    """Fallback for pathological inputs where head/tail capacities overflow:
    all 1600 tokens go through the head gather (13 columns)."""
    import concourse.bacc as bacc

    nc = bacc.Bacc("TRN2", target_bir_lowering=False, debug=False)
    return _build_common(nc, -(-TOK // P), sbuf_stage)


def _to_bf16(a):
    import ml_dtypes

    return np.asarray(a).astype(ml_dtypes.bfloat16)


def _route(ci, cm, pad, head_cols):
    """Per-core token routing. Returns (head_rows[cap padded], src_slot[1600])
    or None if capacities overflow."""
    hcap = head_cols * P
    ent = ci - V
    entc = np.where((ent < 0) | (ent >= N_ENT), N_ENT - 1, ent)
    word = np.where(ci >= V, pad, ci)
    ent_base = V + N_ENT * np.arange(B_LOC)[:, None]
    rows = np.where(cm == 1, ent_base + entc, word)      # [8, 200]
    is_tail = (cm == 1) & (entc == N_ENT - 1)            # [8, 200]

    rows_f = rows.reshape(TOK)
    src_slot = np.empty(TOK, dtype=np.int64)
    t_base = np.arange(L)
    head_toks = []
    for b in range(B_LOC):
        tl = t_base[is_tail[b]]
        if len(tl) > P:  # overflow -> route excess to head
            head_toks.extend((b * L + tl[P:]).tolist())
            tl = tl[:P]
        src_slot[b * L + tl] = (head_cols + b) * P + np.arange(len(tl))
        head_toks.extend((b * L + t_base[~is_tail[b]]).tolist())
    head_toks = np.asarray(sorted(head_toks), dtype=np.int64)
    if len(head_toks) > hcap:
        return None
    src_slot[head_toks] = np.arange(len(head_toks))
    head_rows = np.zeros(hcap, dtype=np.int32)
    head_rows[: len(head_toks)] = rows_f[head_toks]
    return head_rows, src_slot


def _shard_inputs(caption_indices, entities_encoded, word_embedding,
                  pad_token, caption_masks, head_cols):
    ci = np.asarray(caption_indices, dtype=np.int64)          # [64, 200]
    cm = np.asarray(caption_masks, dtype=np.int64)[:, :, 0]   # [64, 200]
    we = _to_bf16(word_embedding)                             # [32000, 512]
    ee = _to_bf16(entities_encoded)                           # [64, 50, 512]
    pad = int(pad_token)

    in_maps, slot_maps = [], []
    for i in range(N_CORES):
        sl = slice(i * B_LOC, (i + 1) * B_LOC)
        routed = _route(ci[sl], cm[sl], pad, head_cols)
        if routed is None:
            return None
        head_rows, src_slot = routed
        if _use_ext() and head_cols == HCOLS:
            # 16-wrap int16 layout for dma_gather: token j -> [j%16, j//16],
            # replicated across the 8 Q7 core groups (128 partitions)
            idx_np = np.ascontiguousarray(
                np.tile(head_rows.astype(np.int16).reshape(-1, 16).T, (8, 1))
            )
        else:
            idx_np = np.ascontiguousarray(
                head_rows.reshape(head_cols, P).T
            )                                                 # [128, hcols]
        tbl = np.concatenate([we, ee[sl].reshape(-1, D)], axis=0)
        in_maps.append({"table": np.ascontiguousarray(tbl), "idx": idx_np})
        slot_maps.append(src_slot)
    return in_maps, slot_maps


LAST_RESULTS = None  # BassKernelResults of the most recent run (for test.py)


def kernel(caption_indices, entities_encoded, word_embedding, pad_token,
           caption_masks):
    global LAST_RESULTS
    from concourse.bass_utils import run_bass_kernel_spmd

    head_cols = HCOLS
    sharded = _shard_inputs(caption_indices, entities_encoded,
                            word_embedding, pad_token, caption_masks,
                            head_cols)
    if sharded is None:
        head_cols = -(-TOK // P)
        sharded = _shard_inputs(caption_indices, entities_encoded,
                                word_embedding, pad_token, caption_masks,
                                head_cols)
        nc = _build_general(_sbuf_stage())
    elif _use_ext():
        nc = _build_ext()
    else:
        nc = _build(_sbuf_stage())
    in_maps, slot_maps = sharded

    res = run_bass_kernel_spmd(
        nc,
        in_maps,
        list(range(N_CORES)),
        trace=bool(os.environ.get("CAPEMB_TRACE")),
    )
    LAST_RESULTS = res
    out = np.empty((B, L, D), dtype=np.float32)
    out_cols = head_cols + TCOLS
    for i in range(N_CORES):
        toks = np.transpose(res.results[i]["out"], (1, 0, 2)).reshape(
            out_cols * P, D
        )
        out[i * B_LOC : (i + 1) * B_LOC] = (
            toks[slot_maps[i]].astype(np.float32).reshape(B_LOC, L, D)
        )
    return out
